# revision 51
# baseline (speedup 1.0000x reference)
"""Causal self-attention on 8 trn2 NeuronCores.

Sharding: data-parallel over batch (2) x tensor-parallel over heads (4/core).
Core c handles batch c//4, heads (c%4)*4 .. (c%4)*4+4.  Each core computes
QKV projection for its heads, causal attention, and a partial c_proj
(y_local @ w_proj[local rows]); the host sums the 4 partials per batch
(b_proj added on host).  Partials return as bf16 (halves output DMA; adds
~1e-3 to rel err, budget 2e-2).

Device kernel notes (v4):
- Matmul inputs are bf16 (host-converted); accumulation stays fp32 in PSUM.
  fp8 was evaluated (DoubleRow would halve QKV/AV) but numpy simulation puts
  every fp8 variant over the 2e-2 gate (qkv 3.3e-2, av 2.4e-2, cproj 3.9e-2).
- Attention uses the S^T = K Q^T orientation so the softmax reduction is a
  matmul: V is augmented with a ones column (col 64), so A@V also yields the
  softmax denominator in psum row 64.
- The two heads of a pair run CONCURRENTLY in the scores matmul via PE row
  tiling (auto tile_position from base partitions 0/64): the second matmul
  of each pair costs ~4ns.
- exp runs without max-subtraction (scores bounded for this problem family).
  One ACTIVATE covers both heads' [128,1024] scores psum.
- Main loop is a fine-grained two-stream schedule: per kt step the PE queue
  gets [fill units, av(kt-2), scores(kt)] so the only stall point is right
  before the ops that need exp(kt-2); projection/cproj work is sliced into
  ~2-matmul fill units drawn from a global generator queue.  This keeps PE
  (the binding engine, ~100us of work) dense while ACT paces the exp stream.
- Input DMAs are ordered by need on one ring: x block 0, wk, wq, bkq first
  (first scores ~8us), then wv/bv/vones/tril, x blocks 1-3, wp.
- PE warm-up: dummy matmuls on a zeroed tile span the initial DMA wait so
  the HAM clock gate (needs ~3.4us sustained activity for 1.2->2.4 GHz)
  lifts before the first real projections.
"""

import os
import sys

for p in ("/root/.axon_site", "/root/.axon_site/_ro/trn_rl_repo", "/root/.axon_site/_ro/pypackages", "/opt/trn_rl_repo"):
    if os.path.isdir(p) and p not in sys.path:
        sys.path.append(p)

import ml_dtypes
import numpy as np

import concourse.bacc as bacc
import concourse.mybir as mybir
import concourse.tile as tile
from concourse.bass_utils import run_bass_kernel_spmd

F32 = mybir.dt.float32
BF16 = mybir.dt.bfloat16
Exp = mybir.ActivationFunctionType.Exp
MULT = mybir.AluOpType.mult
ADD = mybir.AluOpType.add
BF = ml_dtypes.bfloat16

T = 2048            # sequence length (per batch)
C = 1024            # embedding dim
NHL = 4             # heads per core
HD = 64             # head dim
FL = NHL * HD       # local features (256)
CK = C // 128       # contraction chunks (8)
NQG = T // 512      # query groups of 512 (4)
NTT = T // 128      # token tiles of 128 (16)

_CACHE = {}
LAST_RESULTS = None


def _build():
    nc = bacc.Bacc("TRN2", target_bir_lowering=False, debug=False, num_devices=8)

    # x^T arrives block-major ([p, g, ck, 512] image) so each token-block DMA
    # is one fully contiguous descriptor
    x_img = nc.dram_tensor("x_img", [128, CK * T], BF16, kind="ExternalInput").ap()
    # weights arrive pre-arranged as the exact SBUF image (contiguous DMA)
    wk_img = nc.dram_tensor("wk_img", [128, CK * 256], BF16, kind="ExternalInput").ap()
    wq_img = nc.dram_tensor("wq_img", [128, CK * 256], BF16, kind="ExternalInput").ap()
    wv_img = nc.dram_tensor("wv_img", [128, CK * 256], BF16, kind="ExternalInput").ap()
    wp_img = nc.dram_tensor("wp_img", [128, 2 * C], BF16, kind="ExternalInput").ap()
    bkq = nc.dram_tensor("bkq", [128, 4], F32, kind="ExternalInput").ap()
    bv = nc.dram_tensor("bv", [1, FL], F32, kind="ExternalInput").ap()
    tril2 = nc.dram_tensor("tril2", [128, 256], BF16, kind="ExternalInput").ap()
    vones = nc.dram_tensor("vones", [128, NTT * NHL], BF16, kind="ExternalInput").ap()
    out = nc.dram_tensor("out", [T, C], BF16, kind="ExternalOutput").ap()

    with tile.TileContext(nc) as tc:
        with (
            tc.tile_pool(name="persist", bufs=1) as pp,
            tc.tile_pool(name="xpool", bufs=1) as xp,
            tc.tile_pool(name="attp", bufs=8) as ap_,
            tc.tile_pool(name="smallp", bufs=2) as sp,
            tc.tile_pool(name="outp", bufs=2) as op_,
            tc.tile_pool(name="proj_ps", bufs=2, space="PSUM") as pps,
            tc.tile_pool(name="stg_ps", bufs=2, space="PSUM") as sps,
            tc.tile_pool(name="av_ps", bufs=1, space="PSUM") as avps,
        ):
            # ---- persistent SBUF tensors ----
            xt_sb = xp.tile([128, CK * T], BF16)          # 8 chunks of x^T [128, 2048]
            wk_sb = pp.tile([128, CK * 256], BF16, tag="wk")
            wq_sb = pp.tile([128, CK * 256], BF16, tag="wq")
            wv_sb = pp.tile([128, CK * 256], BF16, tag="wv")
            qt_sb = [pp.tile([128, T], BF16, tag=f"qt{p}", name=f"qt{p}") for p in range(2)]
            kt_sb = [pp.tile([128, T], BF16, tag=f"kt{p}", name=f"kt{p}") for p in range(2)]
            v_sb = pp.tile([128, NTT * NHL * (HD + 1)], BF16, tag="v")  # per tile: 4x65
            yt_sb = [pp.tile([128, T], BF16, tag=f"yt{p}", name=f"yt{p}") for p in range(2)]
            wp_sb = pp.tile([128, 2 * C], BF16)
            bkq_sb = pp.tile([128, 4], F32, tag="bkq")
            bv_bc = pp.tile([128, FL], F32, tag="bvbc")
            tril_sb = pp.tile([128, 256], BF16, tag="tril")
            bv_row = pp.tile([1, FL], F32, tag="bvrow")
            vones_sb = pp.tile([128, NTT * NHL], BF16, tag="vones")

            XB = CK * 512  # columns per token block in the block-major image

            def xsl(ck, t0, n):
                """xt_sb slice for contraction chunk ck, tokens [t0, t0+n)."""
                g, j = divmod(t0, 512)
                c0 = g * XB + ck * 512 + j
                return xt_sb[:, c0:c0 + n]

            def xblk(g):
                nc.sync.dma_start(out=xt_sb[:, g * XB:(g + 1) * XB],
                                  in_=x_img[:, g * XB:(g + 1) * XB])

            # PE warm-up across the initial DMA wait (HAM clock gate).
            warm = pp.tile([128, 512], BF16, tag="warm")
            nc.vector.memset(warm[:], 0.0)
            avd = avps.tile([65, 1024], F32, tag="av", name="av_ps")
            for i in range(14):
                nc.tensor.matmul(avd[:, (i % 2) * 512:(i % 2) * 512 + 512],
                                 warm[:, 0:65], warm[:], start=True, stop=True)

            # need-ordered input DMAs, all on one ring (strict FIFO): the
            # per-core DMA bandwidth (~170GB/s with all 8 cores streaming) is
            # the head constraint, so parallel rings only dilute the
            # first-needed bytes; x block 0 is split so the first K/Q chunks
            # start ~1.5us earlier
            nc.sync.dma_start(out=xt_sb[:, 0:XB // 2], in_=x_img[:, 0:XB // 2])
            nc.sync.dma_start(out=wk_sb[:], in_=wk_img)
            nc.sync.dma_start(out=xt_sb[:, XB // 2:XB], in_=x_img[:, XB // 2:XB])
            nc.sync.dma_start(out=wq_sb[:], in_=wq_img)
            nc.sync.dma_start(out=bkq_sb[:], in_=bkq)
            nc.sync.dma_start(out=wv_sb[:], in_=wv_img)
            nc.sync.dma_start(out=bv_row[:], in_=bv)
            nc.sync.dma_start(out=vones_sb[:], in_=vones)
            nc.sync.dma_start(out=tril_sb[:], in_=tril2)
            for g in range(1, NQG):
                xblk(g)
            nc.sync.dma_start(out=wp_sb[:], in_=wp_img)
            nc.gpsimd.partition_broadcast(bv_bc[:], bv_row[:])
            # ones column of the augmented V (col 64 of each head block)
            v_ones = v_sb[:].rearrange("p (n c) -> p n c", c=HD + 1)[:, :, HD]
            nc.vector.tensor_copy(v_ones, vones_sb[:])


            def v_tile(tt):
                return v_sb[:, tt * NHL * (HD + 1):(tt + 1) * NHL * (HD + 1)]

            # ---- fill generators: each yield is ~1-2 matmuls of PE work ----
            def gen_kq(qg, p):
                """K^T and Q^T projections for query group qg, head pair p."""
                qs = qg * 512
                for sel in range(2):  # 0: K, 1: Q
                    w_sb = (wk_sb, wq_sb)[sel]
                    ps = pps.tile([128, 512], F32, tag="proj", name="proj_ps")
                    for ck in range(CK):
                        nc.tensor.matmul(
                            ps[:],
                            w_sb[:, ck * 256 + p * 128: ck * 256 + (p + 1) * 128],
                            xsl(ck, qs, 512),
                            start=(ck == 0), stop=(ck == CK - 1))
                        if ck % 2 == 1:
                            yield
                    dst = (kt_sb, qt_sb)[sel]
                    bcol = (0, 2)[sel]
                    nc.vector.tensor_scalar_add(dst[p][:, qs:qs + 512], ps[:],
                                                bkq_sb[:, bcol + p:bcol + p + 1])
                    yield

            def gen_v(tt):
                """V projection (natural orientation, +bias) for one token tile."""
                ps = pps.tile([128, 512], F32, tag="proj", name="proj_ps")
                for ck in range(CK):
                    nc.tensor.matmul(
                        ps[:, 0:FL],
                        xsl(ck, tt * 128, 128),
                        wv_sb[:, ck * 256:(ck + 1) * 256],
                        start=(ck == 0), stop=(ck == CK - 1))
                    if ck % 4 == 3:
                        yield
                vdst = v_tile(tt).rearrange("p (n c) -> p n c", c=HD + 1)[:, :, 0:HD]
                nc.vector.tensor_tensor(vdst, ps[:, 0:FL].rearrange("p (n c) -> p n c", c=HD),
                                        bv_bc[:].rearrange("p (n c) -> p n c", c=HD), ADD)
                yield

            def gen_cproj(tt, evac="dve", use_stg=False):
                """c_proj partial + output DMA for one 128-token tile.

                evac picks the psum-evacuation engine(s): 'dve' for tiles
                interleaved with attention (ACT time there is exp critical
                path), 'split' for tail tiles (idle ACT + DVE in parallel so
                the proj-psum WAR doesn't stall the next tile's matmuls),
                'act' for tiles overlapping the tail DVE reciprocal chain."""
                ob = op_.tile([128, C], BF16, tag="ob", name="ob")
                ps2 = sps.tile([128, 1024], F32, tag="stg", name="stg_ps") if use_stg else None
                for ng in range(2):
                    if use_stg:
                        ps = ps2[:, ng * 512:(ng + 1) * 512]
                    else:
                        ps = pps.tile([128, 512], F32, tag="proj", name="proj_ps")[:]
                    for f in range(2):
                        nc.tensor.matmul(
                            ps,
                            yt_sb[f][:, tt * 128:(tt + 1) * 128],
                            wp_sb[:, f * C + ng * 512: f * C + ng * 512 + 512],
                            start=(f == 0), stop=(f == 1))
                        yield
                    dst = ob[:, ng * 512:(ng + 1) * 512]
                    if evac == "act" or (evac == "split" and ng == 0):
                        nc.scalar.copy(dst, ps)
                    else:
                        nc.vector.tensor_copy(dst, ps)
                nc.sync.dma_start(out=out[tt * 128:(tt + 1) * 128, :], in_=ob[:])
                yield

            # global fill queue: list of generators, consumed in order
            fills = []
            done_gens = set()

            def pull(n):
                k = 0
                while fills and k < n:
                    try:
                        next(fills[0])
                        k += 1
                    except StopIteration:
                        done_gens.add(id(fills.pop(0)))

            def gate(gen_obj):
                """Pull until gen_obj has been fully consumed (ordering gate)."""
                assert id(gen_obj) in done_gens or gen_obj in fills, \
                    "gated generator was never scheduled"
                while gen_obj in fills:
                    pull(1)

            def drain(gen):
                """Run a generator to completion (barrier fills)."""
                for _ in gen:
                    pass
                done_gens.add(id(gen))

            def emit_attention(qg, p, last=False):
                """Causal attention for query group qg, head pair p.

                Per kt step the PE queue gets [fills, av(kt-2), scores(kt)]:
                av/scores both need exp(kt-2) (psum reuse / att tile), so the
                stall point sits after the independent fill work.  Fill pull
                budget tracks the exp length: full tiles leave ~500ns of PE
                slack per step (2 units), diagonal tiles less (1)."""
                qs = qg * 512
                K = 4 * qg + 4
                assert id(kq[qg, p]) in done_gens, f"kq{(qg, p)} not emitted"
                atts = [None] * K
                av = avps.tile([65, 1024], F32, tag="av", name="av_ps")
                # boundary refill: scores(0) stalls on the stg buffer until
                # the previous group's exp(K-2) finishes reading it; these
                # fills run during that window (and cost nothing when ACT is
                # the laggard, since exp(0) isn't ready then anyway)
                pull(2)

                def emit_scores(kt):
                    d = max(0, (kt - 4 * qg) * 128)
                    stg = sps.tile([128, 1024], F32, tag="stg", name="stg_ps")
                    att = ap_.tile([128, 1024], BF16, tag="att", name="att")
                    atts[kt] = att
                    for h in range(2):
                        nc.tensor.matmul(
                            stg[:, h * 512 + d:(h + 1) * 512],
                            kt_sb[p][h * 64:(h + 1) * 64, kt * 128:(kt + 1) * 128],
                            qt_sb[p][h * 64:(h + 1) * 64, qs + d: qs + 512],
                            start=True, stop=True)
                    # one exp for both heads; on diagonal tiles skip the columns
                    # below the causal offset d via a strided view
                    if d:
                        nc.scalar.activation(
                            att[:].rearrange("q (h j) -> q h j", h=2)[:, :, d:512],
                            stg[:].rearrange("q (h j) -> q h j", h=2)[:, :, d:512],
                            Exp, scale=0.125)
                    else:
                        nc.scalar.activation(att[:], stg[:], Exp, scale=0.125)
                    if kt >= 4 * qg:
                        # in-place causal mask, both heads in one strided op
                        a = att[:].rearrange("q (h j) -> q h j", h=2)[:, :, d:d + 128]
                        m = tril_sb[:].rearrange("q (h j) -> q h j", h=2)
                        nc.vector.tensor_tensor(a, a, m, MULT)

                def emit_av(kt):
                    # the V projection feeding this AV must be fully emitted
                    # (instruction order IS dependency order for the tracker)
                    if vgen.get(kt) is not None:
                        gate(vgen[kt])
                    d = max(0, (kt - 4 * qg) * 128)
                    for h in range(2):
                        nc.tensor.matmul(
                            av[:, h * 512 + d:(h + 1) * 512],
                            v_tile(kt)[:, (2 * p + h) * (HD + 1):(2 * p + h + 1) * (HD + 1)],
                            atts[kt][:, h * 512 + d:(h + 1) * 512],
                            start=(kt == 0), stop=(kt == K - 1))

                # kt processed in pairs: two scores row-tile pairs
                # back-to-back, then two av pairs.  The second av pair then
                # follows a plain matmul instead of a row-tiled pair, dodging
                # the ~120ns array-drain tax the follower of a row-tiled
                # pair pays (att pool is 6 deep to cover the longer lag).
                for kt in range(0, K, 2):
                    emit_scores(kt)
                    emit_scores(kt + 1)
                    if kt % 4 == 0 and kt >= 4:
                        # av quad: only its first pair follows a row-tiled
                        # scores pair (and pays the ~120ns drain tax); the
                        # other three pairs chain off plain matmuls
                        for j in range(kt - 4, kt):
                            emit_av(j)
                    # fills consolidated into the av-quad iterations so the
                    # scores-only iterations chain row-tiled pairs directly
                    # (a pair following a pair dodges the drain tax that a
                    # fill matmul would otherwise pay)
                    if kt % 4 == 0:
                        pull(4 if kt >= 4 * qg else 8)
                for j in range(K - 4, K):
                    emit_av(j)
                if last:
                    # defer normalization to the caller (tail restructure)
                    return av
                # normalization chain (see baseline notes: PSUM partition-64
                # read quirk forces staging the denominator row through SBUF;
                # 1/denom broadcast runs on the otherwise-idle gpsimd).
                dsb = sp.tile([1, 1024], F32, tag="dsb", name="dsb")
                dinv = sp.tile([1, 1024], F32, tag="dinv", name="dinv")
                bc = sp.tile([128, 1024], F32, tag="bc", name="bc")
                # evacuate av quickly (unnormalized) so the next head pair
                # can reuse the av psum banks; scale yt in-place later
                nc.vector.tensor_copy(dsb[:], av[64:65, :])
                nc.vector.reciprocal_approx_fast(out=dinv[:], in_=dsb[:])
                for h in range(2):
                    nc.vector.tensor_copy(yt_sb[p][h * 64:(h + 1) * 64, qs:qs + 512],
                                          av[0:64, h * 512:(h + 1) * 512])
                nc.gpsimd.partition_broadcast(bc[:], dinv[:])
                for h in range(2):
                    y = yt_sb[p][h * 64:(h + 1) * 64, qs:qs + 512]
                    nc.vector.tensor_tensor(
                        y, y, bc[h * 64:(h + 1) * 64, h * 512:(h + 1) * 512], MULT)

            # ---- schedule ----
            # head: kq(0,0) and the first two V tiles gate attention(0,0);
            # everything else trickles through the fill queue, with gates
            # enforcing that each attention's K/Q projections (and the V
            # tiles its AVs read) are emitted before that attention starts.
            kq = {(qg, p): gen_kq(qg, p) for qg in range(NQG) for p in range(2)}
            vgen = {tt: gen_v(tt) for tt in range(NTT)}
            drain(kq[0, 0])
            fills.extend([vgen[0], vgen[1], vgen[2], vgen[3], kq[0, 1], kq[1, 0],
                          vgen[4], vgen[5], vgen[6], vgen[7]])
            emit_attention(0, 0)
            gate(kq[0, 1])
            emit_attention(0, 1)
            gate(kq[1, 0])
            fills.extend([kq[1, 1], vgen[8], vgen[9], vgen[10], vgen[11],
                          kq[2, 0]])
            emit_attention(1, 0)
            gate(kq[1, 1])
            emit_attention(1, 1)
            gate(kq[2, 0])
            fills.extend([kq[2, 1], vgen[12], vgen[13], vgen[14], vgen[15],
                          kq[3, 0], gen_cproj(0), gen_cproj(1)])
            emit_attention(2, 0)
            gate(kq[2, 1])
            fills.extend([gen_cproj(tt) for tt in range(2, 6)])
            emit_attention(2, 1)
            gate(kq[3, 0])
            fills.extend([kq[3, 1], gen_cproj(6), gen_cproj(7)])
            emit_attention(3, 0)
            gate(kq[3, 1])
            fills.extend([gen_cproj(8)])
            avL = emit_attention(3, 1, last=True)
            # ---- tail: deferred last normalization overlapped with cproj ----
            # The serial chain (den copy -> reciprocal -> broadcast -> scale)
            # previously idled the PE ~5.5us (gpsimd broadcast+drain is ~3.4us
            # alone) and let HAM re-throttle.  Instead: reserve cproj(10,11)
            # as PE work during the DVE chain, broadcast 1/den with two tiny
            # PE matmuls (ones[1,64] x dinv[1,512]), and push psum
            # evacuations to the idle ACT.
            qsL = 3 * 512
            dsb = sp.tile([1, 1024], F32, tag="dsb", name="dsb")
            dinv = sp.tile([1, 1024], F32, tag="dinv", name="dinv")
            dinvb = sp.tile([1, 1024], BF16, tag="dinvb", name="dinvb")
            nc.vector.tensor_copy(dsb[0:1, 0:512], avL[64:65, 0:512])
            nc.scalar.copy(dsb[0:1, 512:1024], avL[64:65, 512:1024])
            nc.vector.reciprocal_approx_fast(out=dinv[:], in_=dsb[:])
            nc.vector.tensor_copy(dinvb[:], dinv[:])  # bf16 for the PE bcast
            # unnormalized evacuation of the last group's y on idle ACT
            for h in range(2):
                nc.scalar.copy(yt_sb[1][h * 64:(h + 1) * 64, qsL:qsL + 512],
                               avL[0:64, h * 512:(h + 1) * 512])
            pull(1 << 30)  # finish cproj(8) leftovers
            drain(gen_cproj(9, evac="act"))
            drain(gen_cproj(10, evac="act"))
            drain(gen_cproj(11, evac="act"))
            bct = sps.tile([128, 1024], F32, tag="stg", name="stg_ps")
            for h in range(2):
                nc.tensor.matmul(bct[0:64, h * 512:(h + 1) * 512],
                                 vones_sb[0:1, 0:64],
                                 dinvb[0:1, h * 512:(h + 1) * 512],
                                 start=True, stop=True)
            # normalize the last group's y per 128-token chunk, pipelined
            # with the cproj tile that consumes it
            for i, tt in enumerate(range(12, NTT)):
                c0 = (tt - 12) * 128
                for h in range(2):
                    y = yt_sb[1][h * 64:(h + 1) * 64, tt * 128:(tt + 1) * 128]
                    nc.vector.tensor_tensor(
                        y, y, bct[0:64, h * 512 + c0:h * 512 + c0 + 128], MULT)
                drain(gen_cproj(tt, evac="split"))

    nc.compile()
    return nc


def kernel(x, w_attn, b_attn, w_proj, b_proj):
    global LAST_RESULTS
    x = np.asarray(x, dtype=np.float32)
    w_attn = np.asarray(w_attn, dtype=np.float32)
    b_attn = np.asarray(b_attn, dtype=np.float32)
    w_proj = np.asarray(w_proj, dtype=np.float32)
    b_proj = np.asarray(b_proj, dtype=np.float32)
    b, t, c = x.shape
    assert (b, t, c) == (2, T, C)

    if "nc" not in _CACHE:
        _CACHE["nc"] = _build()
    nc = _CACHE["nc"]

    trilm = np.triu(np.ones((128, 128), dtype=np.float32))  # [k, q]: valid iff k <= q
    in_maps = []
    for core in range(8):
        bi, g = divmod(core, 4)
        cs = FL * g  # column/row offset for this core's 4 heads
        wk = w_attn[:, C + cs:C + cs + FL]
        wq = w_attn[:, cs:cs + FL]
        wv = w_attn[:, 2 * C + cs:2 * C + cs + FL]
        bk = b_attn[C + cs:C + cs + FL]
        bq = b_attn[cs:cs + FL]
        bkq = np.stack([bk[0:128], bk[128:256], bq[0:128], bq[128:256]], axis=1)

        def img(w):  # [C, f] -> SBUF image [128, CK*f] (chunk ck at cols ck*f)
            f = w.shape[1]
            return np.ascontiguousarray(
                w.reshape(CK, 128, f).transpose(1, 0, 2).reshape(128, CK * f)).astype(BF)

        wp_l = w_proj[cs:cs + FL, :]
        in_maps.append({
            "x_img": np.ascontiguousarray(
                x[bi].T.reshape(CK, 128, NQG, 512).transpose(1, 2, 0, 3).reshape(128, CK * T)).astype(BF),
            "wk_img": img(wk),
            "wq_img": img(wq),
            "wv_img": img(wv),
            "wp_img": np.ascontiguousarray(
                wp_l.reshape(2, 128, C).transpose(1, 0, 2).reshape(128, 2 * C)).astype(BF),
            "bkq": np.ascontiguousarray(bkq),
            "bv": np.ascontiguousarray(b_attn[2 * C + cs:2 * C + cs + FL].reshape(1, FL)),
            "tril2": np.tile(trilm, (1, 2)).astype(BF),
            "vones": np.ones((128, NTT * NHL), dtype=BF),
        })

    res = run_bass_kernel_spmd(nc, in_maps, core_ids=list(range(8)))
    LAST_RESULTS = res
    # unshard: sum the 4 tensor-parallel partials of each batch element
    y = np.empty((2, T, C), dtype=np.float32)
    for bi in range(2):
        acc = res.results[4 * bi]["out"].astype(np.float32)
        for g in range(1, 4):
            acc = acc + res.results[4 * bi + g]["out"].astype(np.float32)
        y[bi] = acc + b_proj
    return y


# revision 53
# speedup vs baseline: 1.0199x; 1.0199x over previous
"""Causal self-attention on 8 trn2 NeuronCores.

Sharding: data-parallel over batch (2) x tensor-parallel over heads (4/core).
Core c handles batch c//4, heads (c%4)*4 .. (c%4)*4+4.  Each core computes
QKV projection for its heads, causal attention, and a partial c_proj
(y_local @ w_proj[local rows]); the host sums the 4 partials per batch
(b_proj added on host).  Partials return as bf16 (halves output DMA; adds
~1e-3 to rel err, budget 2e-2).

Device kernel notes (v4):
- Matmul inputs are bf16 (host-converted); accumulation stays fp32 in PSUM.
  fp8 was evaluated (DoubleRow would halve QKV/AV) but numpy simulation puts
  every fp8 variant over the 2e-2 gate (qkv 3.3e-2, av 2.4e-2, cproj 3.9e-2).
- Attention uses the S^T = K Q^T orientation so the softmax reduction is a
  matmul: V is augmented with a ones column (col 64), so A@V also yields the
  softmax denominator in psum row 64.
- The two heads of a pair run CONCURRENTLY in the scores matmul via PE row
  tiling (auto tile_position from base partitions 0/64): the second matmul
  of each pair costs ~4ns.
- exp runs without max-subtraction (scores bounded for this problem family).
  One ACTIVATE covers both heads' [128,1024] scores psum.
- Main loop is a fine-grained two-stream schedule: per kt step the PE queue
  gets [fill units, av(kt-2), scores(kt)] so the only stall point is right
  before the ops that need exp(kt-2); projection/cproj work is sliced into
  ~2-matmul fill units drawn from a global generator queue.  This keeps PE
  (the binding engine, ~100us of work) dense while ACT paces the exp stream.
- Input DMAs are ordered by need on one ring: x block 0, wk, wq, bkq first
  (first scores ~8us), then wv/bv/vones/tril, x blocks 1-3, wp.
- PE warm-up: dummy matmuls on a zeroed tile span the initial DMA wait so
  the HAM clock gate (needs ~3.4us sustained activity for 1.2->2.4 GHz)
  lifts before the first real projections.
"""

import os
import sys

for p in ("/root/.axon_site", "/root/.axon_site/_ro/trn_rl_repo", "/root/.axon_site/_ro/pypackages", "/opt/trn_rl_repo"):
    if os.path.isdir(p) and p not in sys.path:
        sys.path.append(p)

import ml_dtypes
import numpy as np

import concourse.bacc as bacc
import concourse.mybir as mybir
import concourse.tile as tile
from concourse.bass_utils import run_bass_kernel_spmd

F32 = mybir.dt.float32
BF16 = mybir.dt.bfloat16
Exp = mybir.ActivationFunctionType.Exp
MULT = mybir.AluOpType.mult
ADD = mybir.AluOpType.add
BF = ml_dtypes.bfloat16

T = 2048            # sequence length (per batch)
C = 1024            # embedding dim
NHL = 4             # heads per core
HD = 64             # head dim
FL = NHL * HD       # local features (256)
CK = C // 128       # contraction chunks (8)
NQG = T // 512      # query groups of 512 (4)
NTT = T // 128      # token tiles of 128 (16)

_CACHE = {}
LAST_RESULTS = None


def _build():
    nc = bacc.Bacc("TRN2", target_bir_lowering=False, debug=False, num_devices=8)

    # x^T arrives block-major ([p, g, ck, 512] image) so each token-block DMA
    # is one fully contiguous descriptor
    x_img = nc.dram_tensor("x_img", [128, CK * T], BF16, kind="ExternalInput").ap()
    # weights arrive pre-arranged as the exact SBUF image (contiguous DMA)
    wk_img = nc.dram_tensor("wk_img", [128, CK * 256], BF16, kind="ExternalInput").ap()
    wq_img = nc.dram_tensor("wq_img", [128, CK * 256], BF16, kind="ExternalInput").ap()
    wv_img = nc.dram_tensor("wv_img", [128, CK * 256], BF16, kind="ExternalInput").ap()
    wp_img = nc.dram_tensor("wp_img", [128, 2 * C], BF16, kind="ExternalInput").ap()
    bkq = nc.dram_tensor("bkq", [128, 4], F32, kind="ExternalInput").ap()
    bv = nc.dram_tensor("bv", [1, FL], F32, kind="ExternalInput").ap()
    tril2 = nc.dram_tensor("tril2", [128, 256], BF16, kind="ExternalInput").ap()
    vones = nc.dram_tensor("vones", [128, NTT * NHL], BF16, kind="ExternalInput").ap()
    out = nc.dram_tensor("out", [T, C], BF16, kind="ExternalOutput").ap()

    with tile.TileContext(nc) as tc:
        with (
            tc.tile_pool(name="persist", bufs=1) as pp,
            tc.tile_pool(name="xpool", bufs=1) as xp,
            tc.tile_pool(name="attp", bufs=12) as ap_,
            tc.tile_pool(name="smallp", bufs=2) as sp,
            tc.tile_pool(name="outp", bufs=2) as op_,
            tc.tile_pool(name="proj_ps", bufs=2, space="PSUM") as pps,
            tc.tile_pool(name="stg_ps", bufs=2, space="PSUM") as sps,
            tc.tile_pool(name="av_ps", bufs=1, space="PSUM") as avps,
        ):
            # ---- persistent SBUF tensors ----
            xt_sb = xp.tile([128, CK * T], BF16)          # 8 chunks of x^T [128, 2048]
            wk_sb = pp.tile([128, CK * 256], BF16, tag="wk")
            wq_sb = pp.tile([128, CK * 256], BF16, tag="wq")
            wv_sb = pp.tile([128, CK * 256], BF16, tag="wv")
            qt_sb = [pp.tile([128, T], BF16, tag=f"qt{p}", name=f"qt{p}") for p in range(2)]
            kt_sb = [pp.tile([128, T], BF16, tag=f"kt{p}", name=f"kt{p}") for p in range(2)]
            v_sb = pp.tile([128, NTT * NHL * (HD + 1)], BF16, tag="v")  # per tile: 4x65
            yt_sb = [pp.tile([128, T], BF16, tag=f"yt{p}", name=f"yt{p}") for p in range(2)]
            wp_sb = pp.tile([128, 2 * C], BF16)
            bkq_sb = pp.tile([128, 4], F32, tag="bkq")
            bv_bc = pp.tile([128, FL], F32, tag="bvbc")
            tril_sb = pp.tile([128, 256], BF16, tag="tril")
            bv_row = pp.tile([1, FL], F32, tag="bvrow")
            vones_sb = pp.tile([128, NTT * NHL], BF16, tag="vones")

            XB = CK * 512  # columns per token block in the block-major image

            def xsl(ck, t0, n):
                """xt_sb slice for contraction chunk ck, tokens [t0, t0+n)."""
                g, j = divmod(t0, 512)
                c0 = g * XB + ck * 512 + j
                return xt_sb[:, c0:c0 + n]

            def xblk(g):
                nc.sync.dma_start(out=xt_sb[:, g * XB:(g + 1) * XB],
                                  in_=x_img[:, g * XB:(g + 1) * XB])

            # PE warm-up across the initial DMA wait (HAM clock gate).
            warm = pp.tile([128, 512], BF16, tag="warm")
            nc.vector.memset(warm[:], 0.0)
            avd = avps.tile([65, 1024], F32, tag="av", name="av_ps")
            for i in range(14):
                nc.tensor.matmul(avd[:, (i % 2) * 512:(i % 2) * 512 + 512],
                                 warm[:, 0:65], warm[:], start=True, stop=True)

            # need-ordered input DMAs, all on one ring (strict FIFO): the
            # per-core DMA bandwidth (~170GB/s with all 8 cores streaming) is
            # the head constraint, so parallel rings only dilute the
            # first-needed bytes; x block 0 is split so the first K/Q chunks
            # start ~1.5us earlier
            nc.sync.dma_start(out=xt_sb[:, 0:XB // 2], in_=x_img[:, 0:XB // 2])
            nc.sync.dma_start(out=wk_sb[:], in_=wk_img)
            nc.sync.dma_start(out=xt_sb[:, XB // 2:XB], in_=x_img[:, XB // 2:XB])
            nc.sync.dma_start(out=wq_sb[:], in_=wq_img)
            nc.sync.dma_start(out=bkq_sb[:], in_=bkq)
            nc.sync.dma_start(out=wv_sb[:], in_=wv_img)
            nc.sync.dma_start(out=bv_row[:], in_=bv)
            nc.sync.dma_start(out=vones_sb[:], in_=vones)
            nc.sync.dma_start(out=tril_sb[:], in_=tril2)
            for g in range(1, NQG):
                xblk(g)
            nc.sync.dma_start(out=wp_sb[:], in_=wp_img)
            nc.gpsimd.partition_broadcast(bv_bc[:], bv_row[:])
            # ones column of the augmented V (col 64 of each head block)
            v_ones = v_sb[:].rearrange("p (n c) -> p n c", c=HD + 1)[:, :, HD]
            nc.vector.tensor_copy(v_ones, vones_sb[:])


            def v_tile(tt):
                return v_sb[:, tt * NHL * (HD + 1):(tt + 1) * NHL * (HD + 1)]

            # ---- fill generators: each yield is ~1-2 matmuls of PE work ----
            def gen_kq(qg, p):
                """K^T and Q^T projections for query group qg, head pair p."""
                qs = qg * 512
                for sel in range(2):  # 0: K, 1: Q
                    w_sb = (wk_sb, wq_sb)[sel]
                    ps = pps.tile([128, 512], F32, tag="proj", name="proj_ps")
                    for ck in range(CK):
                        nc.tensor.matmul(
                            ps[:],
                            w_sb[:, ck * 256 + p * 128: ck * 256 + (p + 1) * 128],
                            xsl(ck, qs, 512),
                            start=(ck == 0), stop=(ck == CK - 1))
                        if ck % 2 == 1:
                            yield
                    dst = (kt_sb, qt_sb)[sel]
                    bcol = (0, 2)[sel]
                    nc.vector.tensor_scalar_add(dst[p][:, qs:qs + 512], ps[:],
                                                bkq_sb[:, bcol + p:bcol + p + 1])
                    yield

            def gen_v(tt):
                """V projection (natural orientation, +bias) for one token tile."""
                ps = pps.tile([128, 512], F32, tag="proj", name="proj_ps")
                for ck in range(CK):
                    nc.tensor.matmul(
                        ps[:, 0:FL],
                        xsl(ck, tt * 128, 128),
                        wv_sb[:, ck * 256:(ck + 1) * 256],
                        start=(ck == 0), stop=(ck == CK - 1))
                    if ck % 4 == 3:
                        yield
                vdst = v_tile(tt).rearrange("p (n c) -> p n c", c=HD + 1)[:, :, 0:HD]
                nc.vector.tensor_tensor(vdst, ps[:, 0:FL].rearrange("p (n c) -> p n c", c=HD),
                                        bv_bc[:].rearrange("p (n c) -> p n c", c=HD), ADD)
                yield

            def gen_cproj(tt, evac="dve", use_stg=False):
                """c_proj partial + output DMA for one 128-token tile.

                evac picks the psum-evacuation engine(s): 'dve' for tiles
                interleaved with attention (ACT time there is exp critical
                path), 'split' for tail tiles (idle ACT + DVE in parallel so
                the proj-psum WAR doesn't stall the next tile's matmuls),
                'act' for tiles overlapping the tail DVE reciprocal chain."""
                ob = op_.tile([128, C], BF16, tag="ob", name="ob")
                ps2 = sps.tile([128, 1024], F32, tag="stg", name="stg_ps") if use_stg else None
                for ng in range(2):
                    if use_stg:
                        ps = ps2[:, ng * 512:(ng + 1) * 512]
                    else:
                        ps = pps.tile([128, 512], F32, tag="proj", name="proj_ps")[:]
                    for f in range(2):
                        nc.tensor.matmul(
                            ps,
                            yt_sb[f][:, tt * 128:(tt + 1) * 128],
                            wp_sb[:, f * C + ng * 512: f * C + ng * 512 + 512],
                            start=(f == 0), stop=(f == 1))
                        yield
                    dst = ob[:, ng * 512:(ng + 1) * 512]
                    if evac == "act" or (evac == "split" and ng == 0):
                        nc.scalar.copy(dst, ps)
                    else:
                        nc.vector.tensor_copy(dst, ps)
                nc.sync.dma_start(out=out[tt * 128:(tt + 1) * 128, :], in_=ob[:])
                yield

            # global fill queue: list of generators, consumed in order
            fills = []
            done_gens = set()

            def pull(n):
                k = 0
                while fills and k < n:
                    try:
                        next(fills[0])
                        k += 1
                    except StopIteration:
                        done_gens.add(id(fills.pop(0)))

            def gate(gen_obj):
                """Pull until gen_obj has been fully consumed (ordering gate)."""
                assert id(gen_obj) in done_gens or gen_obj in fills, \
                    "gated generator was never scheduled"
                while gen_obj in fills:
                    pull(1)

            def drain(gen):
                """Run a generator to completion (barrier fills)."""
                for _ in gen:
                    pass
                done_gens.add(id(gen))

            def emit_attention(qg, p, last=False):
                """Causal attention for query group qg, head pair p.

                Per kt step the PE queue gets [fills, av(kt-2), scores(kt)]:
                av/scores both need exp(kt-2) (psum reuse / att tile), so the
                stall point sits after the independent fill work.  Fill pull
                budget tracks the exp length: full tiles leave ~500ns of PE
                slack per step (2 units), diagonal tiles less (1)."""
                qs = qg * 512
                K = 4 * qg + 4
                assert id(kq[qg, p]) in done_gens, f"kq{(qg, p)} not emitted"
                atts = [None] * K
                av = avps.tile([65, 1024], F32, tag="av", name="av_ps")
                # boundary refill: scores(0) stalls on the stg buffer until
                # the previous group's exp(K-2) finishes reading it; these
                # fills run during that window (and cost nothing when ACT is
                # the laggard, since exp(0) isn't ready then anyway)
                pull(2)

                def emit_scores(kt):
                    d = max(0, (kt - 4 * qg) * 128)
                    stg = sps.tile([128, 1024], F32, tag="stg", name="stg_ps")
                    att = ap_.tile([128, 1024], BF16, tag="att", name="att")
                    atts[kt] = att
                    for h in range(2):
                        nc.tensor.matmul(
                            stg[:, h * 512 + d:(h + 1) * 512],
                            kt_sb[p][h * 64:(h + 1) * 64, kt * 128:(kt + 1) * 128],
                            qt_sb[p][h * 64:(h + 1) * 64, qs + d: qs + 512],
                            start=True, stop=True)
                    # one exp for both heads; on diagonal tiles skip the columns
                    # below the causal offset d via a strided view
                    if d:
                        nc.scalar.activation(
                            att[:].rearrange("q (h j) -> q h j", h=2)[:, :, d:512],
                            stg[:].rearrange("q (h j) -> q h j", h=2)[:, :, d:512],
                            Exp, scale=0.125)
                    else:
                        nc.scalar.activation(att[:], stg[:], Exp, scale=0.125)
                    if kt >= 4 * qg:
                        # in-place causal mask, both heads in one strided op
                        a = att[:].rearrange("q (h j) -> q h j", h=2)[:, :, d:d + 128]
                        m = tril_sb[:].rearrange("q (h j) -> q h j", h=2)
                        nc.vector.tensor_tensor(a, a, m, MULT)

                def emit_av(kt):
                    # the V projection feeding this AV must be fully emitted
                    # (instruction order IS dependency order for the tracker)
                    if vgen.get(kt) is not None:
                        gate(vgen[kt])
                    d = max(0, (kt - 4 * qg) * 128)
                    for h in range(2):
                        nc.tensor.matmul(
                            av[:, h * 512 + d:(h + 1) * 512],
                            v_tile(kt)[:, (2 * p + h) * (HD + 1):(2 * p + h + 1) * (HD + 1)],
                            atts[kt][:, h * 512 + d:(h + 1) * 512],
                            start=(kt == 0), stop=(kt == K - 1))

                # kt processed in pairs: two scores row-tile pairs
                # back-to-back, then two av pairs.  The second av pair then
                # follows a plain matmul instead of a row-tiled pair, dodging
                # the ~120ns array-drain tax the follower of a row-tiled
                # pair pays (att pool is 6 deep to cover the longer lag).
                next_av = 0
                for kt in range(0, K, 2):
                    emit_scores(kt)
                    emit_scores(kt + 1)
                    if kt % 8 == 0 and kt >= 8:
                        # av octet: only its first pair follows a row-tiled
                        # scores pair (and pays the ~120ns drain tax); the
                        # other seven pairs chain off plain matmuls
                        for j in range(next_av, kt):
                            emit_av(j)
                        next_av = kt
                    pull(2 if kt >= 4 * qg else 4)
                for j in range(next_av, K):
                    emit_av(j)
                if last:
                    # defer normalization to the caller (tail restructure)
                    return av
                # normalization chain (see baseline notes: PSUM partition-64
                # read quirk forces staging the denominator row through SBUF;
                # 1/denom broadcast runs on the otherwise-idle gpsimd).
                dsb = sp.tile([1, 1024], F32, tag="dsb", name="dsb")
                dinv = sp.tile([1, 1024], F32, tag="dinv", name="dinv")
                bc = sp.tile([128, 1024], F32, tag="bc", name="bc")
                # evacuate av quickly (unnormalized) so the next head pair
                # can reuse the av psum banks; scale yt in-place later
                nc.vector.tensor_copy(dsb[:], av[64:65, :])
                nc.vector.reciprocal_approx_fast(out=dinv[:], in_=dsb[:])
                for h in range(2):
                    nc.vector.tensor_copy(yt_sb[p][h * 64:(h + 1) * 64, qs:qs + 512],
                                          av[0:64, h * 512:(h + 1) * 512])
                nc.gpsimd.partition_broadcast(bc[:], dinv[:])
                for h in range(2):
                    y = yt_sb[p][h * 64:(h + 1) * 64, qs:qs + 512]
                    nc.vector.tensor_tensor(
                        y, y, bc[h * 64:(h + 1) * 64, h * 512:(h + 1) * 512], MULT)

            # ---- schedule ----
            # head: kq(0,0) and the first two V tiles gate attention(0,0);
            # everything else trickles through the fill queue, with gates
            # enforcing that each attention's K/Q projections (and the V
            # tiles its AVs read) are emitted before that attention starts.
            kq = {(qg, p): gen_kq(qg, p) for qg in range(NQG) for p in range(2)}
            vgen = {tt: gen_v(tt) for tt in range(NTT)}
            drain(kq[0, 0])
            fills.extend([vgen[0], vgen[1], vgen[2], vgen[3], kq[0, 1], kq[1, 0],
                          vgen[4], vgen[5], vgen[6], vgen[7]])
            emit_attention(0, 0)
            gate(kq[0, 1])
            emit_attention(0, 1)
            gate(kq[1, 0])
            fills.extend([kq[1, 1], vgen[8], vgen[9], vgen[10], vgen[11],
                          kq[2, 0]])
            emit_attention(1, 0)
            gate(kq[1, 1])
            emit_attention(1, 1)
            gate(kq[2, 0])
            fills.extend([kq[2, 1], vgen[12], vgen[13], vgen[14], vgen[15],
                          kq[3, 0], gen_cproj(0), gen_cproj(1)])
            emit_attention(2, 0)
            gate(kq[2, 1])
            fills.extend([gen_cproj(tt) for tt in range(2, 6)])
            emit_attention(2, 1)
            gate(kq[3, 0])
            fills.extend([kq[3, 1], gen_cproj(6), gen_cproj(7)])
            emit_attention(3, 0)
            gate(kq[3, 1])
            fills.extend([gen_cproj(8)])
            avL = emit_attention(3, 1, last=True)
            # ---- tail: deferred last normalization overlapped with cproj ----
            # The serial chain (den copy -> reciprocal -> broadcast -> scale)
            # previously idled the PE ~5.5us (gpsimd broadcast+drain is ~3.4us
            # alone) and let HAM re-throttle.  Instead: reserve cproj(10,11)
            # as PE work during the DVE chain, broadcast 1/den with two tiny
            # PE matmuls (ones[1,64] x dinv[1,512]), and push psum
            # evacuations to the idle ACT.
            qsL = 3 * 512
            dsb = sp.tile([1, 1024], F32, tag="dsb", name="dsb")
            dinv = sp.tile([1, 1024], F32, tag="dinv", name="dinv")
            dinvb = sp.tile([1, 1024], BF16, tag="dinvb", name="dinvb")
            nc.vector.tensor_copy(dsb[0:1, 0:512], avL[64:65, 0:512])
            nc.scalar.copy(dsb[0:1, 512:1024], avL[64:65, 512:1024])
            nc.vector.reciprocal_approx_fast(out=dinv[:], in_=dsb[:])
            nc.vector.tensor_copy(dinvb[:], dinv[:])  # bf16 for the PE bcast
            # unnormalized evacuation of the last group's y on idle ACT
            for h in range(2):
                nc.scalar.copy(yt_sb[1][h * 64:(h + 1) * 64, qsL:qsL + 512],
                               avL[0:64, h * 512:(h + 1) * 512])
            pull(1 << 30)  # finish cproj(8) leftovers
            drain(gen_cproj(9, evac="act"))
            drain(gen_cproj(10, evac="act"))
            drain(gen_cproj(11, evac="act"))
            bct = sps.tile([128, 1024], F32, tag="stg", name="stg_ps")
            for h in range(2):
                nc.tensor.matmul(bct[0:64, h * 512:(h + 1) * 512],
                                 vones_sb[0:1, 0:64],
                                 dinvb[0:1, h * 512:(h + 1) * 512],
                                 start=True, stop=True)
            # normalize the last group's y per 128-token chunk, pipelined
            # with the cproj tile that consumes it
            for i, tt in enumerate(range(12, NTT)):
                c0 = (tt - 12) * 128
                for h in range(2):
                    y = yt_sb[1][h * 64:(h + 1) * 64, tt * 128:(tt + 1) * 128]
                    nc.vector.tensor_tensor(
                        y, y, bct[0:64, h * 512 + c0:h * 512 + c0 + 128], MULT)
                drain(gen_cproj(tt, evac="split"))

    nc.compile()
    return nc


def kernel(x, w_attn, b_attn, w_proj, b_proj):
    global LAST_RESULTS
    x = np.asarray(x, dtype=np.float32)
    w_attn = np.asarray(w_attn, dtype=np.float32)
    b_attn = np.asarray(b_attn, dtype=np.float32)
    w_proj = np.asarray(w_proj, dtype=np.float32)
    b_proj = np.asarray(b_proj, dtype=np.float32)
    b, t, c = x.shape
    assert (b, t, c) == (2, T, C)

    if "nc" not in _CACHE:
        _CACHE["nc"] = _build()
    nc = _CACHE["nc"]

    trilm = np.triu(np.ones((128, 128), dtype=np.float32))  # [k, q]: valid iff k <= q
    in_maps = []
    for core in range(8):
        bi, g = divmod(core, 4)
        cs = FL * g  # column/row offset for this core's 4 heads
        wk = w_attn[:, C + cs:C + cs + FL]
        wq = w_attn[:, cs:cs + FL]
        wv = w_attn[:, 2 * C + cs:2 * C + cs + FL]
        bk = b_attn[C + cs:C + cs + FL]
        bq = b_attn[cs:cs + FL]
        bkq = np.stack([bk[0:128], bk[128:256], bq[0:128], bq[128:256]], axis=1)

        def img(w):  # [C, f] -> SBUF image [128, CK*f] (chunk ck at cols ck*f)
            f = w.shape[1]
            return np.ascontiguousarray(
                w.reshape(CK, 128, f).transpose(1, 0, 2).reshape(128, CK * f)).astype(BF)

        wp_l = w_proj[cs:cs + FL, :]
        in_maps.append({
            "x_img": np.ascontiguousarray(
                x[bi].T.reshape(CK, 128, NQG, 512).transpose(1, 2, 0, 3).reshape(128, CK * T)).astype(BF),
            "wk_img": img(wk),
            "wq_img": img(wq),
            "wv_img": img(wv),
            "wp_img": np.ascontiguousarray(
                wp_l.reshape(2, 128, C).transpose(1, 0, 2).reshape(128, 2 * C)).astype(BF),
            "bkq": np.ascontiguousarray(bkq),
            "bv": np.ascontiguousarray(b_attn[2 * C + cs:2 * C + cs + FL].reshape(1, FL)),
            "tril2": np.tile(trilm, (1, 2)).astype(BF),
            "vones": np.ones((128, NTT * NHL), dtype=BF),
        })

    res = run_bass_kernel_spmd(nc, in_maps, core_ids=list(range(8)))
    LAST_RESULTS = res
    # unshard: sum the 4 tensor-parallel partials of each batch element
    y = np.empty((2, T, C), dtype=np.float32)
    for bi in range(2):
        acc = res.results[4 * bi]["out"].astype(np.float32)
        for g in range(1, 4):
            acc = acc + res.results[4 * bi + g]["out"].astype(np.float32)
        y[bi] = acc + b_proj
    return y
